# revision 65
# baseline (speedup 1.0000x reference)
"""AttentionBlock (GroupNorm -> QKV -> 8-head attention -> proj -> residual)
as a Bass/Tile kernel for Trainium2, data-parallel over batch on 8 cores.

Self-contained: hardcodes shapes B=8, C=512, H=W=32 (N=1024), heads=8, d=64,
groups=32.  Each core processes one batch element; all params replicated.
HW exec ~116.4us in the fast PE p-state, ~138.8us when the device heat-
soaks into its throttled state (259ns vs 216ns per 512-col matmul; the
previous kernel measured 139.7us throttled / ~119us fast).  exec_time =
first non-setup instruction -> end of the ~6.7us framework semaphore
teardown, both included in the graded window.

Where the time goes (fast p-state): PE streams 385 matmuls at the 216ns/
512-col issue floor (~93us active, columns are irreducible: matmuls cannot
cross a PSUM bank boundary so 512 cols is the hard max, and output-size/128
fixes the column count); ACT is ~89us (64 exps of [128,1024] + stats +
evacs) — the two are co-limiting, DVE ~53us, everything else slack.

Design notes:
  * all-bf16 dataflow: x, weights, activations bf16 (host converts); f32
    only in PSUM accumulators and GN statistics.
  * head: x as 4 whole-tile DMAs FIRST in both HW-DGE queue FIFOs (sync +
    scalar; only those two engines + gpsimd-swdge can issue DMAs, and
    per-queue streams run ~120GB/s) with the weights behind them — weight
    tensors are pre-transposed on the HOST into exact SBUF layout so every
    DMA row is one contiguous 3-4KB run (256-byte-packet storms from
    strided layouts starve the x transfer otherwise).  GN stats split
    across engines per tile as it lands: DVE tensor_reduce -> sum(x), ACT
    Square+accum_out -> sum(x^2) (scratch squares land in xn tiles so the
    walrus verifier sees a reader).  One f32 matmul against a host-built
    group-mask matrix (scaled 1/(16*1024)) group-averages AND broadcasts
    mean/E[x^2] back to 128 partitions; var = m2 - mean^2 (mean^2 via ACT
    Square since DVE cannot read PSUM twice in one op).  A warm Sqrt on a
    const AP hoists the ACT table load; GN applies split DVE (t0,t1) /
    ACT Identity-with-scale-bias (t2,t3).
  * pair-blocked QKV weights; K-projection PSUM is evacuated DIRECTLY into
    the zero-padded per-head K tiles (two 64-row bias-adds), no separate k
    staging.  Pair 0's q evac runs on ACT, its k evacs on DVE right after
    the applies, so the first score matmuls aren't gated on one engine.
  * v transposes via ONE dma_start_transpose per head ([64,1024] ->
    [128,(8,64)] 3D out, partition-offset source is fine) straight into
    the persistent vT tiles' v-slots — no PE transposes, no DVE copies.
  * score matmuls contract K=128 against zero-padded per-head K tiles
    (K=64-contraction matmuls produce garbage on real HW; GPSIMD cannot
    touch PSUM; scalar_tensor_tensor doesn't exist on GPSIMD;
    reciprocal_approx_fast inputs must sit at partition offset 0).
  * softmax denominators via the ones-block trick: vT tiles are per-kt
    [64 ones | 64 v] blocks so context rows 0-63 accumulate sum(probs) and
    rows 64-127 the context; they ride the context matmuls for free
    (output rows don't add PE cycles).  Normalize = approx-reciprocal+mul.
  * software pipeline: pair j's scores/exp/context interleave QKV of pair
    j+1 as PE filler; pair 0 leads with its own v projection (lag 5), pair
    3 interleaves its second-half context inline; LAG=3 kt between exp and
    context consumption elsewhere.  Each pair PREFIXES the next pair's
    first 2 score-kts before its own cx2 block, so ACT builds exp
    inventory under the 16-matmul context block instead of starting every
    pair cold (-3.3us: removed all per-kt ACT-wait drips).  PREFIX=2 is
    the max: deeper prefixing needs a pA slot whose WAR release depends on
    a cx2 matmul emitted later (PSUM double-buffer limit).
  * proj: residual folded into the accumulation (identity x x matmul) so
    the evacuation is a copy+bias on ACT — idle at the tail — and DVE
    (busy with pair 3's normalizes) drops out of the tail; out-DMA issues
    split across both HW-DGE queues.
  * PSUM budget 8 banks: pA scores 2x[128,1024] + pX context 2x[128,512]
    + pC staging 2x[128,512].  (Merging the two per-kt exps into one
    [128,2048] ACT op would need 4-bank score tiles x2 bufs and doesn't
    fit; fp8 anywhere in the main path blows the 2e-2 error budget.)
"""

import sys

sys.path.insert(0, "/opt/trn_rl_repo")

import numpy as np
import ml_dtypes

B, C, HH, WW = 8, 512, 32, 32
N = HH * WW          # 1024
NH, HD = 8, 64       # heads, head dim
NG = 32              # groupnorm groups
EPS = 1e-5
NT = C // 128        # 4 channel tiles
KT = N // 128        # 8 key tiles
NP = NH // 2         # 4 head pairs
NCORES = 8
LAG = 3

_CACHE: dict = {}


def _build_program():
    import concourse.bacc as bacc
    import concourse.tile as tile
    from concourse import mybir

    f32 = mybir.dt.float32
    bf16 = mybir.dt.bfloat16
    AF = mybir.ActivationFunctionType
    OP = mybir.AluOpType

    nc = bacc.Bacc("TRN2", target_bir_lowering=False, debug=False)

    x_d = nc.dram_tensor("x", [C, N], bf16, kind="ExternalInput").ap()
    # pair-blocked qkv weights, SBUF layout on host: [pair, cin 128,
    # (ktile,q|k|v) blocks, cout 128] so each partition row is one
    # contiguous 3KB run (big DMA packets).
    wq_d = nc.dram_tensor("wqkvT", [NP, 128, NT * 3 * 128], bf16,
                          kind="ExternalInput").ap()
    wp_d = nc.dram_tensor("wprojT", [128, NT * C], bf16, kind="ExternalInput").ap()
    smalls_d = nc.dram_tensor("smalls", [128, 32], f32, kind="ExternalInput").ap()
    gavg_d = nc.dram_tensor("gavg", [128, 128], f32, kind="ExternalInput").ap()
    ident_d = nc.dram_tensor("ident2", [128, 128], bf16, kind="ExternalInput").ap()
    out_d = nc.dram_tensor("out", [C, N], bf16, kind="ExternalOutput").ap()

    x_dt = x_d.rearrange("(t p) n -> t p n", p=128)
    out_dt = out_d.rearrange("(t p) n -> t p n", p=128)

    from contextlib import ExitStack

    with tile.TileContext(nc) as tc, ExitStack() as ctx:
        sg = ctx.enter_context(tc.tile_pool(name="sg", bufs=1))
        work = ctx.enter_context(tc.tile_pool(name="work", bufs=1))
        pb_pool = ctx.enter_context(tc.tile_pool(name="pbp", bufs=2))
        outp = ctx.enter_context(tc.tile_pool(name="outp", bufs=2))
        rsp = ctx.enter_context(tc.tile_pool(name="rsp", bufs=2))
        # PSUM (8 banks): pA = scores 2x[128,1024]f32 (2 banks each),
        # pX = context accumulators 2x[128,512]f32, pC = staging 2x[128,512]
        pA = ctx.enter_context(tc.tile_pool(name="pA", bufs=2, space="PSUM"))
        pX = ctx.enter_context(tc.tile_pool(name="pX", bufs=2, space="PSUM"))
        pC = ctx.enter_context(tc.tile_pool(name="pC", bufs=2, space="PSUM"))

        # ---- input DMAs: x first in BOTH HW-DGE queue FIFOs (sync +
        # scalar) so weight packets never starve the x transfer; weights
        # split across the two queues behind it.
        x_sb = []
        for t in range(NT):
            x_sb.append(work.tile([128, N], bf16, name=f"x{t}", tag=f"x{t}"))
        nc.sync.dma_start(out=x_sb[0], in_=x_dt[0])
        nc.sync.dma_start(out=x_sb[1], in_=x_dt[1])
        nc.scalar.dma_start(out=x_sb[2], in_=x_dt[2])
        nc.scalar.dma_start(out=x_sb[3], in_=x_dt[3])

        smalls_sb = sg.tile([128, 32], f32, name="smalls_sb")
        nc.sync.dma_start(out=smalls_sb, in_=smalls_d)
        bqkv_sb = smalls_sb[:, 0:12]
        bproj_sb = smalls_sb[:, 12:16]
        gnw_sb = smalls_sb[:, 16:20]
        gnb_sb = smalls_sb[:, 20:24]
        eps_sb = smalls_sb[:, 24:25]
        gavg_sb = sg.tile([128, 128], f32, name="gavg_sb")
        nc.sync.dma_start(out=gavg_sb, in_=gavg_d)
        ident_sb = sg.tile([128, 128], bf16, name="ident_sb")
        nc.sync.dma_start(out=ident_sb, in_=ident_d)

        # warm Sqrt first on the ACT queue so its table load (which also
        # covers Square and Identity) runs before the weight-DMA issues.
        rstd_sb = sg.tile([128, NT], f32, name="rstd_sb")
        one_ap = nc.const_aps.tensor(1.0, (128, 1), f32)
        nc.scalar.activation(out=rstd_sb[:, 0:1], in_=one_ap, func=AF.Sqrt,
                             scale=1.0)

        w_sb = []
        for j in range(NP):
            w_sb.append(
                sg.tile([128, NT * 3 * 128], bf16, name=f"w{j}", tag=f"w{j}")
            )
        nc.sync.dma_start(out=w_sb[0], in_=wq_d[0])
        nc.scalar.dma_start(out=w_sb[1], in_=wq_d[1])
        nc.sync.dma_start(out=w_sb[2], in_=wq_d[2])
        nc.scalar.dma_start(out=w_sb[3], in_=wq_d[3])
        wp_all = sg.tile([128, NT * C], bf16, name="wp_all")
        nc.sync.dma_start(out=wp_all, in_=wp_d)
        wp_sb = [wp_all[:, t * C : (t + 1) * C] for t in range(NT)]

        # persistent vT tiles (per kt a [64 ones | 64 v] block; ones memset
        # once) and zero-padded per-head K tiles.  Memsets run on the idle
        # GPSIMD engine: pair 0's tiles (first consumers) first, the rest
        # AFTER the GN-stats helper op below (gpsimd is in-order).
        vt_sb = [[None, None] for _ in range(NP)]
        kp_sb = [[None, None] for _ in range(NP)]

        def make_pair_tiles(j):
            for h01 in range(2):
                kp = sg.tile([128, N], bf16, name=f"kp{j}_{h01}")
                po = (1 - h01) * HD
                nc.gpsimd.memset(kp[po : po + HD, :], 0.0)
                kp_sb[j][h01] = kp
            for h01 in range(2):
                vt = sg.tile([128, N], bf16, name=f"vt{j}_{h01}")
                nc.gpsimd.memset(
                    vt.rearrange("p (k c) -> p k c", c=128)[:, :, 0:HD], 1.0
                )
                vt_sb[j][h01] = vt

        make_pair_tiles(0)
        make_pair_tiles(1)

        # ---- GroupNorm statistics: DVE sums, ACT sums-of-squares ----
        # (square scratch outputs land in xn tiles that are later
        # overwritten-then-read so the walrus verifier sees a reader.)
        allstats = sg.tile([128, 2 * NT], f32, name="allstats")

        xn_sb = []
        hatt_sb = []
        for t in range(NT):
            xn_sb.append(work.tile([128, N], bf16, name=f"xn{t}", tag=f"xn{t}"))
            hatt_sb.append(work.tile([128, N], bf16, name=f"hatt{t}",
                                     tag=f"hatt{t}"))
        for t in (0, 2, 1, 3):  # expected arrival order (sync/scalar pairs)
            # sum(x) as a tensor_scalar with accum_out (scratch lands in
            # hatt tiles, fully overwritten by the normalizes later)
            nc.vector.tensor_scalar(
                out=hatt_sb[t], in0=x_sb[t], scalar1=1.0, scalar2=0.0,
                op0=OP.mult, op1=OP.add, accum_out=allstats[:, t : t + 1],
            )
            nc.scalar.activation(
                out=xn_sb[t], in_=x_sb[t], func=AF.Square, scale=1.0,
                accum_out=allstats[:, NT + t : NT + t + 1],
            )
        # remaining pairs' kp/vt memsets queue behind the stats helper
        for j in range(2, NP):
            make_pair_tiles(j)

        # one constant f32 matmul (host-precomputed group mask / (16*1024))
        # averages sums over each group AND broadcasts back to all 128
        # partitions: chan_ps = [E[x] per tile | E[x^2] per tile].
        chan_ps = pC.tile([128, 2 * NT], f32, name="chan_ps", tag="acc")
        nc.tensor.matmul(chan_ps, gavg_sb, allstats)
        mean_ps = chan_ps[:, 0:NT]
        m2_ps = chan_ps[:, NT : 2 * NT]
        msq_sb = sg.tile([128, NT], f32, name="msq_sb")
        nc.scalar.activation(out=msq_sb, in_=mean_ps, func=AF.Square, scale=1.0)
        nc.vector.tensor_sub(rstd_sb, m2_ps, msq_sb)  # var
        nc.scalar.activation(
            out=rstd_sb, in_=rstd_sb, func=AF.Sqrt, bias=eps_sb, scale=1.0
        )
        nc.vector.reciprocal(rstd_sb, rstd_sb)
        A_sb = sg.tile([128, NT], f32, name="A_sb")
        nc.vector.tensor_mul(A_sb, rstd_sb, gnw_sb)
        B_sb = sg.tile([128, NT], f32, name="B_sb")
        nc.vector.tensor_mul(B_sb, mean_ps, A_sb)
        nc.vector.tensor_sub(B_sb, gnb_sb, B_sb)

        # applies split across DVE (t0,t1) and ACT Identity (t2,t3) so the
        # first QKV matmuls aren't gated on one engine's serial stream.
        for hlf in range(2):
            for t in range(NT):
                src = x_sb[t][:, hlf * 512 : (hlf + 1) * 512]
                dst = xn_sb[t][:, hlf * 512 : (hlf + 1) * 512]
                if t < 2:
                    nc.vector.tensor_scalar(
                        out=dst, in0=src,
                        scalar1=A_sb[:, t : t + 1],
                        scalar2=B_sb[:, t : t + 1],
                        op0=OP.mult, op1=OP.add,
                    )
                else:
                    nc.scalar.activation(
                        out=dst, in_=src, func=AF.Identity,
                        bias=B_sb[:, t : t + 1], scale=A_sb[:, t : t + 1],
                    )

        # ---- QKV / transpose building blocks ----
        q_sb = [None] * NP
        v_sb = [None] * NP

        def qkv_half(j, ty, hlf):
            """One m-half of pair j's q/k/v projection (4 matmuls + evac).
            K (ty==1) evacuates straight into the zero-padded per-head
            tiles as two 64-row bias-adds."""
            acc = pC.tile([128, 512], f32, name=f"acc{j}_{ty}_{hlf}", tag="acc")
            for kc in range(NT):
                nc.tensor.matmul(
                    acc,
                    w_sb[j][:, (kc * 3 + ty) * 128 : (kc * 3 + ty + 1) * 128],
                    xn_sb[kc][:, hlf * 512 : (hlf + 1) * 512],
                    start=(kc == 0),
                    stop=(kc == NT - 1),
                )
            bias = bqkv_sb[:, j * 3 + ty : j * 3 + ty + 1]
            # pair 0's q evacuation runs on ACT (idle until the first exp)
            # while its k evacuations take DVE right after the applies —
            # splitting them lets pair 0's score stream start earliest.
            use_act = j == 0 and ty == 0

            def evac(dst, src, b):
                if use_act:
                    nc.scalar.activation(
                        out=dst, in_=src, func=AF.Identity, bias=b, scale=1.0
                    )
                else:
                    nc.vector.tensor_scalar_add(dst, src, b)

            if ty == 1:
                for h01 in range(2):
                    po = h01 * HD
                    evac(
                        kp_sb[j][h01][po : po + HD, hlf * 512 : (hlf + 1) * 512],
                        acc[po : po + HD, :],
                        bias[po : po + HD, :],
                    )
            else:
                dest = (q_sb, None, v_sb)[ty]
                if dest[j] is None:
                    dest[j] = work.tile(
                        [128, N], bf16, name=f"qkv{j}_{ty}", tag=f"qkv{j}_{ty}"
                    )
                evac(dest[j][:, hlf * 512 : (hlf + 1) * 512], acc, bias)

        def vtrans_pair(j, cr=None):
            """Both heads' v transposed into the persistent vT tiles' v-slots
            by the DMA transpose XBAR (sync HW-DGE queue) — no PE or DVE
            involvement.  cr=(k0,k1) restricts to a kt range so pair 0 can
            transpose the kts covered by each v-half as soon as it lands."""
            k0, k1 = (0, KT) if cr is None else cr
            for h01 in range(2):
                nc.sync.dma_start_transpose(
                    out=vt_sb[j][h01]
                    .rearrange("p (k c) -> p k c", c=128)[:, k0:k1, HD:128],
                    in_=v_sb[j][h01 * HD : (h01 + 1) * HD, k0 * 128 : k1 * 128],
                )

        # ---- attention pair with interleaved filler ----
        # pbs_all[j]: exp outputs per pair, shared so a pair's first score
        # kts can be emitted ("prefixed") during the PREVIOUS pair's cx2
        # block — ACT builds exp inventory while the PE streams context,
        # instead of starting every pair's exp stream cold.
        PREFIX = 2
        pbs_all = [dict() for _ in range(NP)]

        def emit_sc(j, kt):
            for h01 in range(2):
                sc = pA.tile([128, N], f32, name=f"sc{j}_{h01}_{kt}", tag="sc")
                for hlf in range(2):
                    nc.tensor.matmul(
                        sc[:, hlf * 512 : (hlf + 1) * 512],
                        kp_sb[j][h01][:, kt * 128 : (kt + 1) * 128],
                        q_sb[j][:, hlf * 512 : (hlf + 1) * 512],
                    )
                pb = pb_pool.tile(
                    [128, N], bf16, name=f"pb{h01}_{kt}", tag=f"pb{h01}_{kt}"
                )
                nc.scalar.activation(out=pb, in_=sc, func=AF.Exp, scale=0.125)
                pbs_all[j][(h01, kt)] = pb

        def attn_pair(j):
            last = j == NP - 1
            lag = 5 if j == 0 else LAG
            pbs = pbs_all[j]
            cx1 = {}
            cx2 = {}
            # filler units: pair 0 leads with its own v projection +
            # transpose (so its exp stream starts right after q/k);
            # pairs 0-2 then carry pair j+1's qkv halves + v transposes.
            filler = []
            if j == 0:
                filler.append(lambda: qkv_half(0, 2, 0))
                filler.append(lambda: vtrans_pair(0, (0, KT // 2)))
                filler.append(lambda: qkv_half(0, 2, 1))
                filler.append(lambda: vtrans_pair(0, (KT // 2, KT)))
            if not last:
                jn = j + 1
                for ty in range(3):
                    for hlf in range(2):
                        filler.append(lambda ty=ty, hlf=hlf: qkv_half(jn, ty, hlf))
                filler.append(lambda: vtrans_pair(jn))

            def emit_cx1(kt):
                for h01 in range(2):
                    if kt == 0:
                        cx1[h01] = pX.tile(
                            [128, 512], f32, name=f"cx1_{h01}", tag="cx"
                        )
                    nc.tensor.matmul(
                        cx1[h01],
                        vt_sb[j][h01][:, kt * 128 : (kt + 1) * 128],
                        pbs[(h01, kt)][:, 0:512],
                        start=(kt == 0),
                        stop=(kt == KT - 1),
                    )

            def emit_cx2(kt):
                for h01 in range(2):
                    if kt == 0:
                        pool, tag = (pC, "acc") if last else (pX, "cx")
                        cx2[h01] = pool.tile(
                            [128, 512], f32, name=f"cx2_{h01}", tag=tag
                        )
                    nc.tensor.matmul(
                        cx2[h01],
                        vt_sb[j][h01][:, kt * 128 : (kt + 1) * 128],
                        pbs[(h01, kt)][:, 512:1024],
                        start=(kt == 0),
                        stop=(kt == KT - 1),
                    )

            def normalize(cx, hlf):
                for h01 in range(2):
                    rsb = rsp.tile([HD, 512], f32, name=f"rs{h01}", tag="rs")
                    nc.vector.reciprocal_approx_fast(out=rsb, in_=cx[h01][0:HD, :])
                    nc.vector.tensor_mul(
                        hatt_sb[j][h01 * HD : (h01 + 1) * HD,
                                   hlf * 512 : (hlf + 1) * 512],
                        cx[h01][HD:128, :],
                        rsb,
                    )

            # kt loop: scores + exp lead; first-half context LAGs; filler
            # (next pair's qkv) keeps the PE fed while ACT drains the exp
            # backlog.  Pair 3 interleaves second-half context instead.
            # Pairs 1-3 skip their first PREFIX kts (emitted by the
            # previous pair, below).
            fi = 0
            start = 0 if j == 0 else PREFIX
            for kt in range(start, KT):
                emit_sc(j, kt)
                if kt >= lag:
                    emit_cx1(kt - lag)
                    if last:
                        emit_cx2(kt - lag)
                if (kt > 0 or j > 0) and fi < len(filler):
                    filler[fi]()
                    fi += 1
            for kt in range(KT - lag, KT):
                emit_cx1(kt)
                if last:
                    emit_cx2(kt)
            while fi < len(filler):
                filler[fi]()
                fi += 1
            # prefix the NEXT pair's first score kts here, so its exps run
            # on ACT underneath our cx2 block / normalizes.
            if not last:
                for pkt in range(PREFIX):
                    emit_sc(j + 1, pkt)
            normalize(cx1, 0)
            if not last:
                for kt in range(KT):
                    emit_cx2(kt)
            normalize(cx2, 1)

        for ty in range(2):
            for hlf in range(2):
                qkv_half(0, ty, hlf)
        for j in range(NP):
            attn_pair(j)

        # ---- proj + bias + residual ----
        # residual folded into the accumulation (identity x x), so the
        # evacuation is a copy+bias on ACT — which is idle at the tail —
        # and DVE (busy with pair 3's normalizes) drops out entirely.
        for mt in range(NT):
            ot = outp.tile([128, N], bf16, name=f"ot{mt}", tag="ot")
            for hlf in range(2):
                # rotate accumulator pools by m-tile, earliest-freed first
                ppool, ptag = [(pA, "sc"), (pX, "cx"), (pC, "acc"), (pA, "sc")][mt]
                pp = ppool.tile([128, 512], f32, name=f"pp{mt}_{hlf}", tag=ptag)
                nc.tensor.matmul(
                    pp,
                    ident_sb,
                    x_sb[mt][:, hlf * 512 : (hlf + 1) * 512],
                    start=True,
                    stop=False,
                )
                for kc in range(NT):
                    nc.tensor.matmul(
                        pp,
                        wp_sb[kc][:, mt * 128 : (mt + 1) * 128],
                        hatt_sb[kc][:, hlf * 512 : (hlf + 1) * 512],
                        start=False,
                        stop=(kc == NT - 1),
                    )
                nc.scalar.activation(
                    out=ot[:, hlf * 512 : (hlf + 1) * 512],
                    in_=pp, func=AF.Identity,
                    bias=bproj_sb[:, mt : mt + 1], scale=1.0,
                )
                if mt == NT - 1:
                    # the final tile's transfers gate the end of the
                    # program: split each half across BOTH HW-DGE queues
                    c = hlf * 512
                    nc.scalar.dma_start(
                        out=out_dt[mt][:, c : c + 256],
                        in_=ot[:, c : c + 256],
                    )
                    nc.sync.dma_start(
                        out=out_dt[mt][:, c + 256 : c + 512],
                        in_=ot[:, c + 256 : c + 512],
                    )
                else:
                    dq = nc.scalar if hlf == 0 else nc.sync
                    dq.dma_start(
                        out=out_dt[mt][:, hlf * 512 : (hlf + 1) * 512],
                        in_=ot[:, hlf * 512 : (hlf + 1) * 512],
                    )

    nc.compile()
    return nc


def _get_nc():
    if "nc" not in _CACHE:
        _CACHE["nc"] = _build_program()
    return _CACHE["nc"]


def _host_inputs(x, gn_w, gn_b, qkv_w, qkv_b, proj_w, proj_b):
    f32 = np.float32
    bf = ml_dtypes.bfloat16
    x = np.asarray(x, dtype=f32).reshape(B, C, N)
    gn_w = np.asarray(gn_w, dtype=f32)
    gn_b = np.asarray(gn_b, dtype=f32)
    qkv_w = np.asarray(qkv_w, dtype=f32)
    qkv_b = np.asarray(qkv_b, dtype=f32)
    proj_w = np.asarray(proj_w, dtype=f32)
    proj_b = np.asarray(proj_b, dtype=f32)

    # pair-blocked qkv weights in SBUF layout: [pair, cin 128,
    # (ktile,q|k|v)*cout] — contiguous per partition row for big packets.
    wq = np.zeros((NP, 128, NT * 3, 128), f32)
    bq = np.zeros((128, NP * 3), f32)
    for j in range(NP):
        for ty in range(3):
            rows = qkv_w[ty * 512 + j * 128 : ty * 512 + (j + 1) * 128]  # [128, C]
            for kt in range(NT):
                wq[j, :, kt * 3 + ty, :] = rows[:, kt * 128 : (kt + 1) * 128].T
            bq[:, j * 3 + ty] = qkv_b[ty * 512 + j * 128 : ty * 512 + (j + 1) * 128]
    wq = wq.reshape(NP, 128, NT * 3 * 128)
    # proj weights in SBUF layout: [cin 128, ktile*cout]
    wp = np.ascontiguousarray(
        proj_w.T.reshape(NT, 128, C).transpose(1, 0, 2).reshape(128, NT * C)
    )

    bproj = np.ascontiguousarray(proj_b.reshape(NT, 128).T)
    gnw = np.ascontiguousarray(gn_w.reshape(NT, 128).T)
    gnb = np.ascontiguousarray(gn_b.reshape(NT, 128).T)

    epscol = np.full((128, 1), EPS, f32)
    pad = np.zeros((128, 7), f32)

    gavg = np.ascontiguousarray(
        ((np.arange(128)[:, None] // 16) == (np.arange(128)[None, :] // 16))
        .astype(f32) / (16.0 * N)
    )

    smalls = np.concatenate([bq, bproj, gnw, gnb, epscol, pad], axis=1)
    assert smalls.shape == (128, 32)

    ident2 = np.ascontiguousarray(np.eye(128, dtype=f32))

    common = dict(
        wqkvT=wq.astype(bf), wprojT=wp.astype(bf),
        smalls=np.ascontiguousarray(smalls), gavg=gavg,
        ident2=ident2.astype(bf),
    )
    return [
        dict(common, x=np.ascontiguousarray(x[b]).astype(bf)) for b in range(B)
    ]


def _run(in_maps, trace=False, **kw):
    from concourse.bass_utils import run_bass_kernel_spmd

    nc = _get_nc()
    return run_bass_kernel_spmd(nc, in_maps, list(range(NCORES)), trace=trace, **kw)


def kernel(x, gn_w, gn_b, qkv_w, qkv_b, proj_w, proj_b):
    in_maps = _host_inputs(x, gn_w, gn_b, qkv_w, qkv_b, proj_w, proj_b)
    res = _run(in_maps)
    out = np.stack(
        [np.asarray(res.results[b]["out"]).astype(np.float32) for b in range(B)]
    )
    return out.reshape(B, C, HH, WW)


# revision 68
# speedup vs baseline: 1.0107x; 1.0107x over previous
"""AttentionBlock (GroupNorm -> QKV -> 8-head attention -> proj -> residual)
as a Bass/Tile kernel for Trainium2, data-parallel over batch on 8 cores.

Self-contained: hardcodes shapes B=8, C=512, H=W=32 (N=1024), heads=8, d=64,
groups=32.  Each core processes one batch element; all params replicated.
HW exec ~116.4us in the fast PE p-state, ~138.8us when the device heat-
soaks into its throttled state (259ns vs 216ns per 512-col matmul; the
previous kernel measured 139.7us throttled / ~119us fast).  exec_time =
first non-setup instruction -> end of the ~6.7us framework semaphore
teardown, both included in the graded window.

Where the time goes (fast p-state): PE streams 385 matmuls at the 216ns/
512-col issue floor (~93us active, columns are irreducible: matmuls cannot
cross a PSUM bank boundary so 512 cols is the hard max, and output-size/128
fixes the column count); ACT is ~89us (64 exps of [128,1024] + stats +
evacs) — the two are co-limiting, DVE ~53us, everything else slack.

Design notes:
  * all-bf16 dataflow: x, weights, activations bf16 (host converts); f32
    only in PSUM accumulators and GN statistics.
  * head: x as 4 whole-tile DMAs FIRST in both HW-DGE queue FIFOs (sync +
    scalar; only those two engines + gpsimd-swdge can issue DMAs, and
    per-queue streams run ~120GB/s) with the weights behind them — weight
    tensors are pre-transposed on the HOST into exact SBUF layout so every
    DMA row is one contiguous 3-4KB run (256-byte-packet storms from
    strided layouts starve the x transfer otherwise).  GN stats split
    across engines per tile as it lands: DVE tensor_reduce -> sum(x), ACT
    Square+accum_out -> sum(x^2) (scratch squares land in xn tiles so the
    walrus verifier sees a reader).  One f32 matmul against a host-built
    group-mask matrix (scaled 1/(16*1024)) group-averages AND broadcasts
    mean/E[x^2] back to 128 partitions; var = m2 - mean^2 (mean^2 via ACT
    Square since DVE cannot read PSUM twice in one op).  A warm Sqrt on a
    const AP hoists the ACT table load; GN applies split DVE (t0,t1) /
    ACT Identity-with-scale-bias (t2,t3).
  * pair-blocked QKV weights; K-projection PSUM is evacuated DIRECTLY into
    the zero-padded per-head K tiles (two 64-row bias-adds), no separate k
    staging.  Pair 0's q evac runs on ACT, its k evacs on DVE right after
    the applies, so the first score matmuls aren't gated on one engine.
  * v transposes via ONE dma_start_transpose per head ([64,1024] ->
    [128,(8,64)] 3D out, partition-offset source is fine) straight into
    the persistent vT tiles' v-slots — no PE transposes, no DVE copies.
  * score matmuls contract K=128 against zero-padded per-head K tiles
    (K=64-contraction matmuls produce garbage on real HW; GPSIMD cannot
    touch PSUM; scalar_tensor_tensor doesn't exist on GPSIMD;
    reciprocal_approx_fast inputs must sit at partition offset 0).
  * softmax denominators via the ones-block trick: vT tiles are per-kt
    [64 ones | 64 v] blocks so context rows 0-63 accumulate sum(probs) and
    rows 64-127 the context; they ride the context matmuls for free
    (output rows don't add PE cycles).  Normalize = approx-reciprocal+mul.
  * software pipeline: pair j's scores/exp/context interleave QKV of pair
    j+1 as PE filler; pair 0 leads with its own v projection (lag 5), pair
    3 interleaves its second-half context inline; LAG=3 kt between exp and
    context consumption elsewhere.  Each pair PREFIXES the next pair's
    first 2 score-kts before its own cx2 block, so ACT builds exp
    inventory under the 16-matmul context block instead of starting every
    pair cold (-3.3us: removed all per-kt ACT-wait drips).  PREFIX=2 is
    the max: deeper prefixing needs a pA slot whose WAR release depends on
    a cx2 matmul emitted later (PSUM double-buffer limit).
  * proj: residual folded into the accumulation (identity x x matmul) so
    the evacuation is a copy+bias on ACT — idle at the tail — and DVE
    (busy with pair 3's normalizes) drops out of the tail; out-DMA issues
    split across both HW-DGE queues.
  * PSUM budget 8 banks: pA scores 2x[128,1024] + pX context 2x[128,512]
    + pC staging 2x[128,512].  (Merging the two per-kt exps into one
    [128,2048] ACT op would need 4-bank score tiles x2 bufs and doesn't
    fit; fp8 anywhere in the main path blows the 2e-2 error budget.)
"""

import sys

sys.path.insert(0, "/opt/trn_rl_repo")

import numpy as np
import ml_dtypes

B, C, HH, WW = 8, 512, 32, 32
N = HH * WW          # 1024
NH, HD = 8, 64       # heads, head dim
NG = 32              # groupnorm groups
EPS = 1e-5
NT = C // 128        # 4 channel tiles
KT = N // 128        # 8 key tiles
NP = NH // 2         # 4 head pairs
NCORES = 8
LAG = 3

_CACHE: dict = {}


def _build_program():
    import concourse.bacc as bacc
    import concourse.tile as tile
    from concourse import mybir

    f32 = mybir.dt.float32
    bf16 = mybir.dt.bfloat16
    AF = mybir.ActivationFunctionType
    OP = mybir.AluOpType

    nc = bacc.Bacc("TRN2", target_bir_lowering=False, debug=False)

    x_d = nc.dram_tensor("x", [C, N], bf16, kind="ExternalInput").ap()
    # pair-blocked qkv weights, SBUF layout on host: [pair, cin 128,
    # (ktile,q|k|v) blocks, cout 128] so each partition row is one
    # contiguous 3KB run (big DMA packets).
    wq_d = nc.dram_tensor("wqkvT", [NP, 128, NT * 3 * 128], bf16,
                          kind="ExternalInput").ap()
    wp_d = nc.dram_tensor("wprojT", [128, NT * C], bf16, kind="ExternalInput").ap()
    smalls_d = nc.dram_tensor("smalls", [128, 32], f32, kind="ExternalInput").ap()
    gavg_d = nc.dram_tensor("gavg", [128, 128], f32, kind="ExternalInput").ap()
    ident_d = nc.dram_tensor("ident2", [128, 128], bf16, kind="ExternalInput").ap()
    out_d = nc.dram_tensor("out", [C, N], bf16, kind="ExternalOutput").ap()

    x_dt = x_d.rearrange("(t p) n -> t p n", p=128)
    out_dt = out_d.rearrange("(t p) n -> t p n", p=128)

    from contextlib import ExitStack

    with tile.TileContext(nc) as tc, ExitStack() as ctx:
        sg = ctx.enter_context(tc.tile_pool(name="sg", bufs=1))
        work = ctx.enter_context(tc.tile_pool(name="work", bufs=1))
        pb_pool = ctx.enter_context(tc.tile_pool(name="pbp", bufs=2))
        outp = ctx.enter_context(tc.tile_pool(name="outp", bufs=2))
        rsp = ctx.enter_context(tc.tile_pool(name="rsp", bufs=2))
        # PSUM (8 banks): pA = scores 2x[128,1024]f32 (2 banks each),
        # pX = context accumulators 2x[128,512]f32, pC = staging 2x[128,512]
        pA = ctx.enter_context(tc.tile_pool(name="pA", bufs=2, space="PSUM"))
        pX = ctx.enter_context(tc.tile_pool(name="pX", bufs=2, space="PSUM"))
        pC = ctx.enter_context(tc.tile_pool(name="pC", bufs=2, space="PSUM"))

        # ---- input DMAs: x first in BOTH HW-DGE queue FIFOs (sync +
        # scalar) so weight packets never starve the x transfer; weights
        # split across the two queues behind it.
        x_sb = []
        for t in range(NT):
            x_sb.append(work.tile([128, N], bf16, name=f"x{t}", tag=f"x{t}"))
        nc.sync.dma_start(out=x_sb[0], in_=x_dt[0])
        nc.sync.dma_start(out=x_sb[1], in_=x_dt[1])
        nc.scalar.dma_start(out=x_sb[2], in_=x_dt[2])
        nc.scalar.dma_start(out=x_sb[3], in_=x_dt[3])

        smalls_sb = sg.tile([128, 32], f32, name="smalls_sb")
        nc.sync.dma_start(out=smalls_sb, in_=smalls_d)
        bqkv_sb = smalls_sb[:, 0:12]
        bproj_sb = smalls_sb[:, 12:16]
        gnw_sb = smalls_sb[:, 16:20]
        gnb_sb = smalls_sb[:, 20:24]
        eps_sb = smalls_sb[:, 24:25]
        gavg_sb = sg.tile([128, 128], f32, name="gavg_sb")
        nc.sync.dma_start(out=gavg_sb, in_=gavg_d)
        ident_sb = sg.tile([128, 128], bf16, name="ident_sb")
        nc.sync.dma_start(out=ident_sb, in_=ident_d)

        # warm Sqrt first on the ACT queue so its table load (which also
        # covers Square and Identity) runs before the weight-DMA issues.
        rstd_sb = sg.tile([128, NT], f32, name="rstd_sb")
        one_ap = nc.const_aps.tensor(1.0, (128, 1), f32)
        nc.scalar.activation(out=rstd_sb[:, 0:1], in_=one_ap, func=AF.Sqrt,
                             scale=1.0)

        w_sb = []
        for j in range(NP):
            w_sb.append(
                sg.tile([128, NT * 3 * 128], bf16, name=f"w{j}", tag=f"w{j}")
            )
        nc.sync.dma_start(out=w_sb[0], in_=wq_d[0])
        nc.scalar.dma_start(out=w_sb[1], in_=wq_d[1])
        nc.sync.dma_start(out=w_sb[2], in_=wq_d[2])
        nc.scalar.dma_start(out=w_sb[3], in_=wq_d[3])
        wp_all = sg.tile([128, NT * C], bf16, name="wp_all")
        nc.sync.dma_start(out=wp_all, in_=wp_d)
        wp_sb = [wp_all[:, t * C : (t + 1) * C] for t in range(NT)]

        # persistent vT tiles (per kt a [64 ones | 64 v] block; ones memset
        # once) and zero-padded per-head K tiles.  Memsets run on the idle
        # GPSIMD engine: pair 0's tiles (first consumers) first, the rest
        # AFTER the GN-stats helper op below (gpsimd is in-order).
        vt_sb = [[None, None] for _ in range(NP)]
        kp_sb = [[None, None] for _ in range(NP)]

        def make_pair_tiles(j):
            for h01 in range(2):
                kp = sg.tile([128, N], bf16, name=f"kp{j}_{h01}")
                po = (1 - h01) * HD
                nc.gpsimd.memset(kp[po : po + HD, :], 0.0)
                kp_sb[j][h01] = kp
            for h01 in range(2):
                vt = sg.tile([128, N], bf16, name=f"vt{j}_{h01}")
                nc.gpsimd.memset(
                    vt.rearrange("p (k c) -> p k c", c=128)[:, :, 0:HD], 1.0
                )
                vt_sb[j][h01] = vt

        make_pair_tiles(0)
        make_pair_tiles(1)

        # ---- GroupNorm statistics: DVE sums, ACT sums-of-squares ----
        # (square scratch outputs land in xn tiles that are later
        # overwritten-then-read so the walrus verifier sees a reader.)
        allstats = sg.tile([128, 2 * NT], f32, name="allstats")

        xn_sb = []
        hatt_sb = []
        for t in range(NT):
            xn_sb.append(work.tile([128, N], bf16, name=f"xn{t}", tag=f"xn{t}"))
            hatt_sb.append(work.tile([128, N], bf16, name=f"hatt{t}",
                                     tag=f"hatt{t}"))
        for t in (0, 2, 1, 3):  # expected arrival order (sync/scalar pairs)
            nc.vector.tensor_reduce(
                out=allstats[:, t : t + 1], in_=x_sb[t],
                axis=mybir.AxisListType.X, op=OP.add,
            )
            nc.scalar.activation(
                out=xn_sb[t], in_=x_sb[t], func=AF.Square, scale=1.0,
                accum_out=allstats[:, NT + t : NT + t + 1],
            )
        # remaining pairs' kp/vt memsets queue behind the stats helper
        for j in range(2, NP):
            make_pair_tiles(j)

        # one constant f32 matmul (host-precomputed group mask / (16*1024))
        # averages sums over each group AND broadcasts back to all 128
        # partitions: chan_ps = [E[x] per tile | E[x^2] per tile].
        chan_ps = pC.tile([128, 2 * NT], f32, name="chan_ps", tag="acc")
        nc.tensor.matmul(chan_ps, gavg_sb, allstats)
        mean_ps = chan_ps[:, 0:NT]
        m2_ps = chan_ps[:, NT : 2 * NT]
        msq_sb = sg.tile([128, NT], f32, name="msq_sb")
        nc.scalar.activation(out=msq_sb, in_=mean_ps, func=AF.Square, scale=1.0)
        nc.vector.tensor_sub(rstd_sb, m2_ps, msq_sb)  # var
        nc.scalar.activation(
            out=rstd_sb, in_=rstd_sb, func=AF.Sqrt, bias=eps_sb, scale=1.0
        )
        nc.vector.reciprocal(rstd_sb, rstd_sb)
        A_sb = sg.tile([128, NT], f32, name="A_sb")
        nc.vector.tensor_mul(A_sb, rstd_sb, gnw_sb)
        B_sb = sg.tile([128, NT], f32, name="B_sb")
        nc.vector.tensor_mul(B_sb, mean_ps, A_sb)
        nc.vector.tensor_sub(B_sb, gnb_sb, B_sb)

        # applies split across DVE (t0,t1) and ACT Identity (t2,t3) so the
        # first QKV matmuls aren't gated on one engine's serial stream.
        for hlf in range(2):
            for t in range(NT):
                src = x_sb[t][:, hlf * 512 : (hlf + 1) * 512]
                dst = xn_sb[t][:, hlf * 512 : (hlf + 1) * 512]
                if t < 2:
                    nc.vector.tensor_scalar(
                        out=dst, in0=src,
                        scalar1=A_sb[:, t : t + 1],
                        scalar2=B_sb[:, t : t + 1],
                        op0=OP.mult, op1=OP.add,
                    )
                else:
                    nc.scalar.activation(
                        out=dst, in_=src, func=AF.Identity,
                        bias=B_sb[:, t : t + 1], scale=A_sb[:, t : t + 1],
                    )

        # ---- QKV / transpose building blocks ----
        q_sb = [None] * NP
        v_sb = [None] * NP

        def qkv_half(j, ty, hlf):
            """One m-half of pair j's q/k/v projection (4 matmuls + evac).
            K (ty==1) evacuates straight into the zero-padded per-head
            tiles as two 64-row bias-adds."""
            acc = pC.tile([128, 512], f32, name=f"acc{j}_{ty}_{hlf}", tag="acc")
            for kc in range(NT):
                nc.tensor.matmul(
                    acc,
                    w_sb[j][:, (kc * 3 + ty) * 128 : (kc * 3 + ty + 1) * 128],
                    xn_sb[kc][:, hlf * 512 : (hlf + 1) * 512],
                    start=(kc == 0),
                    stop=(kc == NT - 1),
                )
            bias = bqkv_sb[:, j * 3 + ty : j * 3 + ty + 1]
            # pair 0's q evacuation runs on ACT (idle until the first exp)
            # while its k evacuations take DVE right after the applies —
            # splitting them lets pair 0's score stream start earliest.
            use_act = j == 0 and ty == 0

            def evac(dst, src, b):
                if use_act:
                    nc.scalar.activation(
                        out=dst, in_=src, func=AF.Identity, bias=b, scale=1.0
                    )
                else:
                    nc.vector.tensor_scalar_add(dst, src, b)

            if ty == 1:
                for h01 in range(2):
                    po = h01 * HD
                    evac(
                        kp_sb[j][h01][po : po + HD, hlf * 512 : (hlf + 1) * 512],
                        acc[po : po + HD, :],
                        bias[po : po + HD, :],
                    )
            else:
                dest = (q_sb, None, v_sb)[ty]
                if dest[j] is None:
                    dest[j] = work.tile(
                        [128, N], bf16, name=f"qkv{j}_{ty}", tag=f"qkv{j}_{ty}"
                    )
                evac(dest[j][:, hlf * 512 : (hlf + 1) * 512], acc, bias)

        def vtrans_pair(j, cr=None):
            """Both heads' v transposed into the persistent vT tiles' v-slots
            by the DMA transpose XBAR (sync HW-DGE queue) — no PE or DVE
            involvement.  cr=(k0,k1) restricts to a kt range so pair 0 can
            transpose the kts covered by each v-half as soon as it lands."""
            k0, k1 = (0, KT) if cr is None else cr
            for h01 in range(2):
                nc.sync.dma_start_transpose(
                    out=vt_sb[j][h01]
                    .rearrange("p (k c) -> p k c", c=128)[:, k0:k1, HD:128],
                    in_=v_sb[j][h01 * HD : (h01 + 1) * HD, k0 * 128 : k1 * 128],
                )

        # ---- attention pair with interleaved filler ----
        # pbs_all[j]: exp outputs per pair, shared so a pair's first score
        # kts can be emitted ("prefixed") during the PREVIOUS pair's cx2
        # block — ACT builds exp inventory while the PE streams context,
        # instead of starting every pair's exp stream cold.
        PREFIX = 2
        pbs_all = [dict() for _ in range(NP)]

        def emit_sc(j, kt):
            for h01 in range(2):
                sc = pA.tile([128, N], f32, name=f"sc{j}_{h01}_{kt}", tag="sc")
                for hlf in range(2):
                    nc.tensor.matmul(
                        sc[:, hlf * 512 : (hlf + 1) * 512],
                        kp_sb[j][h01][:, kt * 128 : (kt + 1) * 128],
                        q_sb[j][:, hlf * 512 : (hlf + 1) * 512],
                    )
                pb = pb_pool.tile(
                    [128, N], bf16, name=f"pb{h01}_{kt}", tag=f"pb{h01}_{kt}"
                )
                nc.scalar.activation(out=pb, in_=sc, func=AF.Exp, scale=0.125)
                pbs_all[j][(h01, kt)] = pb

        def attn_pair(j):
            last = j == NP - 1
            lag = 5 if j == 0 else LAG
            pbs = pbs_all[j]
            cx1 = {}
            cx2 = {}
            # filler units: pair 0 leads with its own v projection +
            # transpose (so its exp stream starts right after q/k);
            # pairs 0-2 then carry pair j+1's qkv halves + v transposes.
            filler = []
            if j == 0:
                for hlf in range(2):
                    filler.append(lambda hlf=hlf: qkv_half(0, 2, hlf))
                filler.append(lambda: vtrans_pair(0))
            if not last:
                jn = j + 1
                for ty in range(3):
                    for hlf in range(2):
                        filler.append(lambda ty=ty, hlf=hlf: qkv_half(jn, ty, hlf))
                filler.append(lambda: vtrans_pair(jn))

            def emit_cx1(kt):
                for h01 in range(2):
                    if kt == 0:
                        cx1[h01] = pX.tile(
                            [128, 512], f32, name=f"cx1_{h01}", tag="cx"
                        )
                    nc.tensor.matmul(
                        cx1[h01],
                        vt_sb[j][h01][:, kt * 128 : (kt + 1) * 128],
                        pbs[(h01, kt)][:, 0:512],
                        start=(kt == 0),
                        stop=(kt == KT - 1),
                    )

            def emit_cx2(kt):
                for h01 in range(2):
                    if kt == 0:
                        pool, tag = (pC, "acc") if last else (pX, "cx")
                        cx2[h01] = pool.tile(
                            [128, 512], f32, name=f"cx2_{h01}", tag=tag
                        )
                    nc.tensor.matmul(
                        cx2[h01],
                        vt_sb[j][h01][:, kt * 128 : (kt + 1) * 128],
                        pbs[(h01, kt)][:, 512:1024],
                        start=(kt == 0),
                        stop=(kt == KT - 1),
                    )

            def normalize(cx, hlf):
                for h01 in range(2):
                    rsb = rsp.tile([HD, 512], f32, name=f"rs{h01}", tag="rs")
                    nc.vector.reciprocal_approx_fast(out=rsb, in_=cx[h01][0:HD, :])
                    nc.vector.tensor_mul(
                        hatt_sb[j][h01 * HD : (h01 + 1) * HD,
                                   hlf * 512 : (hlf + 1) * 512],
                        cx[h01][HD:128, :],
                        rsb,
                    )

            # kt loop: scores + exp lead; first-half context LAGs; filler
            # (next pair's qkv) keeps the PE fed while ACT drains the exp
            # backlog.  Pair 3 interleaves second-half context instead.
            # Pairs 1-3 skip their first PREFIX kts (emitted by the
            # previous pair, below).
            fi = 0
            start = 0 if j == 0 else PREFIX
            for kt in range(start, KT):
                emit_sc(j, kt)
                if kt >= lag:
                    emit_cx1(kt - lag)
                    if last:
                        emit_cx2(kt - lag)
                if (kt > 0 or j > 0) and fi < len(filler):
                    filler[fi]()
                    fi += 1
            for kt in range(KT - lag, KT):
                emit_cx1(kt)
                if last:
                    emit_cx2(kt)
            while fi < len(filler):
                filler[fi]()
                fi += 1
            # prefix the NEXT pair's first score kts here, so its exps run
            # on ACT underneath our cx2 block / normalizes.
            if not last:
                for pkt in range(PREFIX):
                    emit_sc(j + 1, pkt)
            normalize(cx1, 0)
            if not last:
                for kt in range(KT):
                    emit_cx2(kt)
            normalize(cx2, 1)

        for ty in range(2):
            for hlf in range(2):
                qkv_half(0, ty, hlf)
        for j in range(NP):
            attn_pair(j)

        # ---- proj + bias + residual ----
        # residual folded into the accumulation (identity x x), so the
        # evacuation is a copy+bias on ACT — which is idle at the tail —
        # and DVE (busy with pair 3's normalizes) drops out entirely.
        for mt in range(NT):
            ot = outp.tile([128, N], bf16, name=f"ot{mt}", tag="ot")
            for hlf in range(2):
                # rotate accumulator pools by m-tile, earliest-freed first
                ppool, ptag = [(pA, "sc"), (pX, "cx"), (pC, "acc"), (pA, "sc")][mt]
                pp = ppool.tile([128, 512], f32, name=f"pp{mt}_{hlf}", tag=ptag)
                nc.tensor.matmul(
                    pp,
                    ident_sb,
                    x_sb[mt][:, hlf * 512 : (hlf + 1) * 512],
                    start=True,
                    stop=False,
                )
                for kc in range(NT):
                    nc.tensor.matmul(
                        pp,
                        wp_sb[kc][:, mt * 128 : (mt + 1) * 128],
                        hatt_sb[kc][:, hlf * 512 : (hlf + 1) * 512],
                        start=False,
                        stop=(kc == NT - 1),
                    )
                nc.scalar.activation(
                    out=ot[:, hlf * 512 : (hlf + 1) * 512],
                    in_=pp, func=AF.Identity,
                    bias=bproj_sb[:, mt : mt + 1], scale=1.0,
                )
                dq = nc.scalar if hlf == 0 else nc.sync
                dq.dma_start(
                    out=out_dt[mt][:, hlf * 512 : (hlf + 1) * 512],
                    in_=ot[:, hlf * 512 : (hlf + 1) * 512],
                )

    nc.compile()
    return nc


def _get_nc():
    if "nc" not in _CACHE:
        _CACHE["nc"] = _build_program()
    return _CACHE["nc"]


def _host_inputs(x, gn_w, gn_b, qkv_w, qkv_b, proj_w, proj_b):
    f32 = np.float32
    bf = ml_dtypes.bfloat16
    x = np.asarray(x, dtype=f32).reshape(B, C, N)
    gn_w = np.asarray(gn_w, dtype=f32)
    gn_b = np.asarray(gn_b, dtype=f32)
    qkv_w = np.asarray(qkv_w, dtype=f32)
    qkv_b = np.asarray(qkv_b, dtype=f32)
    proj_w = np.asarray(proj_w, dtype=f32)
    proj_b = np.asarray(proj_b, dtype=f32)

    # pair-blocked qkv weights in SBUF layout: [pair, cin 128,
    # (ktile,q|k|v)*cout] — contiguous per partition row for big packets.
    wq = np.zeros((NP, 128, NT * 3, 128), f32)
    bq = np.zeros((128, NP * 3), f32)
    for j in range(NP):
        for ty in range(3):
            rows = qkv_w[ty * 512 + j * 128 : ty * 512 + (j + 1) * 128]  # [128, C]
            for kt in range(NT):
                wq[j, :, kt * 3 + ty, :] = rows[:, kt * 128 : (kt + 1) * 128].T
            bq[:, j * 3 + ty] = qkv_b[ty * 512 + j * 128 : ty * 512 + (j + 1) * 128]
    wq = wq.reshape(NP, 128, NT * 3 * 128)
    # proj weights in SBUF layout: [cin 128, ktile*cout]
    wp = np.ascontiguousarray(
        proj_w.T.reshape(NT, 128, C).transpose(1, 0, 2).reshape(128, NT * C)
    )

    bproj = np.ascontiguousarray(proj_b.reshape(NT, 128).T)
    gnw = np.ascontiguousarray(gn_w.reshape(NT, 128).T)
    gnb = np.ascontiguousarray(gn_b.reshape(NT, 128).T)

    epscol = np.full((128, 1), EPS, f32)
    pad = np.zeros((128, 7), f32)

    gavg = np.ascontiguousarray(
        ((np.arange(128)[:, None] // 16) == (np.arange(128)[None, :] // 16))
        .astype(f32) / (16.0 * N)
    )

    smalls = np.concatenate([bq, bproj, gnw, gnb, epscol, pad], axis=1)
    assert smalls.shape == (128, 32)

    ident2 = np.ascontiguousarray(np.eye(128, dtype=f32))

    common = dict(
        wqkvT=wq.astype(bf), wprojT=wp.astype(bf),
        smalls=np.ascontiguousarray(smalls), gavg=gavg,
        ident2=ident2.astype(bf),
    )
    return [
        dict(common, x=np.ascontiguousarray(x[b]).astype(bf)) for b in range(B)
    ]


def _run(in_maps, trace=False, **kw):
    from concourse.bass_utils import run_bass_kernel_spmd

    nc = _get_nc()
    return run_bass_kernel_spmd(nc, in_maps, list(range(NCORES)), trace=trace, **kw)


def kernel(x, gn_w, gn_b, qkv_w, qkv_b, proj_w, proj_b):
    in_maps = _host_inputs(x, gn_w, gn_b, qkv_w, qkv_b, proj_w, proj_b)
    res = _run(in_maps)
    out = np.stack(
        [np.asarray(res.results[b]["out"]).astype(np.float32) for b in range(B)]
    )
    return out.reshape(B, C, HH, WW)


# revision 69
# speedup vs baseline: 1.0114x; 1.0007x over previous
"""AttentionBlock (GroupNorm -> QKV -> 8-head attention -> proj -> residual)
as a Bass/Tile kernel for Trainium2, data-parallel over batch on 8 cores.

Self-contained: hardcodes shapes B=8, C=512, H=W=32 (N=1024), heads=8, d=64,
groups=32.  Each core processes one batch element; all params replicated.
HW exec ~116.4us in the fast PE p-state, ~138.8us when the device heat-
soaks into its throttled state (259ns vs 216ns per 512-col matmul; the
previous kernel measured 139.7us throttled / ~119us fast).  exec_time =
first non-setup instruction -> end of the ~6.7us framework semaphore
teardown, both included in the graded window.

Where the time goes (fast p-state): PE streams 385 matmuls at the 216ns/
512-col issue floor (~93us active, columns are irreducible: matmuls cannot
cross a PSUM bank boundary so 512 cols is the hard max, and output-size/128
fixes the column count); ACT is ~89us (64 exps of [128,1024] + stats +
evacs) — the two are co-limiting, DVE ~53us, everything else slack.

Design notes:
  * all-bf16 dataflow: x, weights, activations bf16 (host converts); f32
    only in PSUM accumulators and GN statistics.
  * head: x as 4 whole-tile DMAs FIRST in both HW-DGE queue FIFOs (sync +
    scalar; only those two engines + gpsimd-swdge can issue DMAs, and
    per-queue streams run ~120GB/s) with the weights behind them — weight
    tensors are pre-transposed on the HOST into exact SBUF layout so every
    DMA row is one contiguous 3-4KB run (256-byte-packet storms from
    strided layouts starve the x transfer otherwise).  GN stats split
    across engines per tile as it lands: DVE tensor_reduce -> sum(x), ACT
    Square+accum_out -> sum(x^2) (scratch squares land in xn tiles so the
    walrus verifier sees a reader).  One f32 matmul against a host-built
    group-mask matrix (scaled 1/(16*1024)) group-averages AND broadcasts
    mean/E[x^2] back to 128 partitions; var = m2 - mean^2 (mean^2 via ACT
    Square since DVE cannot read PSUM twice in one op).  A warm Sqrt on a
    const AP hoists the ACT table load; GN applies split DVE (t0,t1) /
    ACT Identity-with-scale-bias (t2,t3).
  * pair-blocked QKV weights; K-projection PSUM is evacuated DIRECTLY into
    the zero-padded per-head K tiles (two 64-row bias-adds), no separate k
    staging.  Pair 0's q evac runs on ACT, its k evacs on DVE right after
    the applies, so the first score matmuls aren't gated on one engine.
  * v transposes via ONE dma_start_transpose per head ([64,1024] ->
    [128,(8,64)] 3D out, partition-offset source is fine) straight into
    the persistent vT tiles' v-slots — no PE transposes, no DVE copies.
  * score matmuls contract K=128 against zero-padded per-head K tiles
    (K=64-contraction matmuls produce garbage on real HW; GPSIMD cannot
    touch PSUM; scalar_tensor_tensor doesn't exist on GPSIMD;
    reciprocal_approx_fast inputs must sit at partition offset 0).
  * softmax denominators via the ones-block trick: vT tiles are per-kt
    [64 ones | 64 v] blocks so context rows 0-63 accumulate sum(probs) and
    rows 64-127 the context; they ride the context matmuls for free
    (output rows don't add PE cycles).  Normalize = approx-reciprocal+mul.
  * software pipeline: pair j's scores/exp/context interleave QKV of pair
    j+1 as PE filler; pair 0 leads with its own v projection (lag 5), pair
    3 interleaves its second-half context inline; LAG=3 kt between exp and
    context consumption elsewhere.  Each pair PREFIXES the next pair's
    first 2 score-kts before its own cx2 block, so ACT builds exp
    inventory under the 16-matmul context block instead of starting every
    pair cold (-3.3us: removed all per-kt ACT-wait drips).  PREFIX=2 is
    the max: deeper prefixing needs a pA slot whose WAR release depends on
    a cx2 matmul emitted later (PSUM double-buffer limit).
  * proj: residual folded into the accumulation (identity x x matmul) so
    the evacuation is a copy+bias on ACT — idle at the tail — and DVE
    (busy with pair 3's normalizes) drops out of the tail; out-DMA issues
    split across both HW-DGE queues.
  * PSUM budget 8 banks: pA scores 2x[128,1024] + pX context 2x[128,512]
    + pC staging 2x[128,512].  (Merging the two per-kt exps into one
    [128,2048] ACT op would need 4-bank score tiles x2 bufs and doesn't
    fit; fp8 anywhere in the main path blows the 2e-2 error budget.)
"""

import sys

sys.path.insert(0, "/opt/trn_rl_repo")

import numpy as np
import ml_dtypes

B, C, HH, WW = 8, 512, 32, 32
N = HH * WW          # 1024
NH, HD = 8, 64       # heads, head dim
NG = 32              # groupnorm groups
EPS = 1e-5
NT = C // 128        # 4 channel tiles
KT = N // 128        # 8 key tiles
NP = NH // 2         # 4 head pairs
NCORES = 8
LAG = 3

_CACHE: dict = {}


def _build_program():
    import concourse.bacc as bacc
    import concourse.tile as tile
    from concourse import mybir

    f32 = mybir.dt.float32
    bf16 = mybir.dt.bfloat16
    AF = mybir.ActivationFunctionType
    OP = mybir.AluOpType

    nc = bacc.Bacc("TRN2", target_bir_lowering=False, debug=False)

    x_d = nc.dram_tensor("x", [C, N], bf16, kind="ExternalInput").ap()
    # pair-blocked qkv weights, SBUF layout on host: [pair, cin 128,
    # (ktile,q|k|v) blocks, cout 128] so each partition row is one
    # contiguous 3KB run (big DMA packets).
    wq_d = nc.dram_tensor("wqkvT", [NP, 128, NT * 3 * 128], bf16,
                          kind="ExternalInput").ap()
    wp_d = nc.dram_tensor("wprojT", [128, NT * C], bf16, kind="ExternalInput").ap()
    smalls_d = nc.dram_tensor("smalls", [128, 32], f32, kind="ExternalInput").ap()
    gavg_d = nc.dram_tensor("gavg", [128, 128], f32, kind="ExternalInput").ap()
    ident_d = nc.dram_tensor("ident2", [128, 128], bf16, kind="ExternalInput").ap()
    out_d = nc.dram_tensor("out", [C, N], bf16, kind="ExternalOutput").ap()

    x_dt = x_d.rearrange("(t p) n -> t p n", p=128)
    out_dt = out_d.rearrange("(t p) n -> t p n", p=128)

    from contextlib import ExitStack

    with tile.TileContext(nc) as tc, ExitStack() as ctx:
        sg = ctx.enter_context(tc.tile_pool(name="sg", bufs=1))
        work = ctx.enter_context(tc.tile_pool(name="work", bufs=1))
        pb_pool = ctx.enter_context(tc.tile_pool(name="pbp", bufs=2))
        outp = ctx.enter_context(tc.tile_pool(name="outp", bufs=2))
        rsp = ctx.enter_context(tc.tile_pool(name="rsp", bufs=2))
        # PSUM (8 banks): pA = scores 2x[128,1024]f32 (2 banks each),
        # pX = context accumulators 2x[128,512]f32, pC = staging 2x[128,512]
        pA = ctx.enter_context(tc.tile_pool(name="pA", bufs=2, space="PSUM"))
        pX = ctx.enter_context(tc.tile_pool(name="pX", bufs=2, space="PSUM"))
        pC = ctx.enter_context(tc.tile_pool(name="pC", bufs=2, space="PSUM"))

        # ---- input DMAs: x first in BOTH HW-DGE queue FIFOs (sync +
        # scalar) so weight packets never starve the x transfer; weights
        # split across the two queues behind it.
        x_sb = []
        for t in range(NT):
            x_sb.append(work.tile([128, N], bf16, name=f"x{t}", tag=f"x{t}"))
        nc.sync.dma_start(out=x_sb[0], in_=x_dt[0])
        nc.sync.dma_start(out=x_sb[1], in_=x_dt[1])
        nc.scalar.dma_start(out=x_sb[2], in_=x_dt[2])
        nc.scalar.dma_start(out=x_sb[3], in_=x_dt[3])

        smalls_sb = sg.tile([128, 32], f32, name="smalls_sb")
        nc.sync.dma_start(out=smalls_sb, in_=smalls_d)
        bqkv_sb = smalls_sb[:, 0:12]
        bproj_sb = smalls_sb[:, 12:16]
        gnw_sb = smalls_sb[:, 16:20]
        gnb_sb = smalls_sb[:, 20:24]
        eps_sb = smalls_sb[:, 24:25]
        gavg_sb = sg.tile([128, 128], f32, name="gavg_sb")
        nc.sync.dma_start(out=gavg_sb, in_=gavg_d)
        ident_sb = sg.tile([128, 128], bf16, name="ident_sb")
        nc.sync.dma_start(out=ident_sb, in_=ident_d)

        # warm Sqrt first on the ACT queue so its table load (which also
        # covers Square and Identity) runs before the weight-DMA issues.
        rstd_sb = sg.tile([128, NT], f32, name="rstd_sb")
        one_ap = nc.const_aps.tensor(1.0, (128, 1), f32)
        nc.scalar.activation(out=rstd_sb[:, 0:1], in_=one_ap, func=AF.Sqrt,
                             scale=1.0)

        w_sb = []
        for j in range(NP):
            w_sb.append(
                sg.tile([128, NT * 3 * 128], bf16, name=f"w{j}", tag=f"w{j}")
            )
        # ALL weight DMAs go through the sync queue: issue instructions on
        # the scalar (ACT) engine would sit between the warm-up and the
        # first Square in ACT's in-order queue, delaying the GN stats
        # chain ~1us past x0's arrival.  The sync stream still lands every
        # weight well before its first consumer (w0 ~16us vs needed ~19;
        # wp ~30us vs needed ~105).
        for j in range(NP):
            nc.sync.dma_start(out=w_sb[j], in_=wq_d[j])
        wp_all = sg.tile([128, NT * C], bf16, name="wp_all")
        nc.sync.dma_start(out=wp_all, in_=wp_d)
        wp_sb = [wp_all[:, t * C : (t + 1) * C] for t in range(NT)]

        # persistent vT tiles (per kt a [64 ones | 64 v] block; ones memset
        # once) and zero-padded per-head K tiles.  Memsets run on the idle
        # GPSIMD engine: pair 0's tiles (first consumers) first, the rest
        # AFTER the GN-stats helper op below (gpsimd is in-order).
        vt_sb = [[None, None] for _ in range(NP)]
        kp_sb = [[None, None] for _ in range(NP)]

        def make_pair_tiles(j):
            for h01 in range(2):
                kp = sg.tile([128, N], bf16, name=f"kp{j}_{h01}")
                po = (1 - h01) * HD
                nc.gpsimd.memset(kp[po : po + HD, :], 0.0)
                kp_sb[j][h01] = kp
            for h01 in range(2):
                vt = sg.tile([128, N], bf16, name=f"vt{j}_{h01}")
                nc.gpsimd.memset(
                    vt.rearrange("p (k c) -> p k c", c=128)[:, :, 0:HD], 1.0
                )
                vt_sb[j][h01] = vt

        make_pair_tiles(0)
        make_pair_tiles(1)

        # ---- GroupNorm statistics: DVE sums, ACT sums-of-squares ----
        # (square scratch outputs land in xn tiles that are later
        # overwritten-then-read so the walrus verifier sees a reader.)
        allstats = sg.tile([128, 2 * NT], f32, name="allstats")

        xn_sb = []
        hatt_sb = []
        for t in range(NT):
            xn_sb.append(work.tile([128, N], bf16, name=f"xn{t}", tag=f"xn{t}"))
            hatt_sb.append(work.tile([128, N], bf16, name=f"hatt{t}",
                                     tag=f"hatt{t}"))
        for t in (0, 2, 1, 3):  # expected arrival order (sync/scalar pairs)
            nc.vector.tensor_reduce(
                out=allstats[:, t : t + 1], in_=x_sb[t],
                axis=mybir.AxisListType.X, op=OP.add,
            )
            nc.scalar.activation(
                out=xn_sb[t], in_=x_sb[t], func=AF.Square, scale=1.0,
                accum_out=allstats[:, NT + t : NT + t + 1],
            )
        # remaining pairs' kp/vt memsets queue behind the stats helper
        for j in range(2, NP):
            make_pair_tiles(j)

        # one constant f32 matmul (host-precomputed group mask / (16*1024))
        # averages sums over each group AND broadcasts back to all 128
        # partitions: chan_ps = [E[x] per tile | E[x^2] per tile].
        chan_ps = pC.tile([128, 2 * NT], f32, name="chan_ps", tag="acc")
        nc.tensor.matmul(chan_ps, gavg_sb, allstats)
        mean_ps = chan_ps[:, 0:NT]
        m2_ps = chan_ps[:, NT : 2 * NT]
        msq_sb = sg.tile([128, NT], f32, name="msq_sb")
        nc.scalar.activation(out=msq_sb, in_=mean_ps, func=AF.Square, scale=1.0)
        nc.vector.tensor_sub(rstd_sb, m2_ps, msq_sb)  # var
        nc.scalar.activation(
            out=rstd_sb, in_=rstd_sb, func=AF.Sqrt, bias=eps_sb, scale=1.0
        )
        nc.vector.reciprocal(rstd_sb, rstd_sb)
        A_sb = sg.tile([128, NT], f32, name="A_sb")
        nc.vector.tensor_mul(A_sb, rstd_sb, gnw_sb)
        B_sb = sg.tile([128, NT], f32, name="B_sb")
        nc.vector.tensor_mul(B_sb, mean_ps, A_sb)
        nc.vector.tensor_sub(B_sb, gnb_sb, B_sb)

        # applies split across DVE (t0,t1) and ACT Identity (t2,t3) so the
        # first QKV matmuls aren't gated on one engine's serial stream.
        for hlf in range(2):
            for t in range(NT):
                src = x_sb[t][:, hlf * 512 : (hlf + 1) * 512]
                dst = xn_sb[t][:, hlf * 512 : (hlf + 1) * 512]
                if t < 2:
                    nc.vector.tensor_scalar(
                        out=dst, in0=src,
                        scalar1=A_sb[:, t : t + 1],
                        scalar2=B_sb[:, t : t + 1],
                        op0=OP.mult, op1=OP.add,
                    )
                else:
                    nc.scalar.activation(
                        out=dst, in_=src, func=AF.Identity,
                        bias=B_sb[:, t : t + 1], scale=A_sb[:, t : t + 1],
                    )

        # ---- QKV / transpose building blocks ----
        q_sb = [None] * NP
        v_sb = [None] * NP

        def qkv_half(j, ty, hlf):
            """One m-half of pair j's q/k/v projection (4 matmuls + evac).
            K (ty==1) evacuates straight into the zero-padded per-head
            tiles as two 64-row bias-adds."""
            acc = pC.tile([128, 512], f32, name=f"acc{j}_{ty}_{hlf}", tag="acc")
            for kc in range(NT):
                nc.tensor.matmul(
                    acc,
                    w_sb[j][:, (kc * 3 + ty) * 128 : (kc * 3 + ty + 1) * 128],
                    xn_sb[kc][:, hlf * 512 : (hlf + 1) * 512],
                    start=(kc == 0),
                    stop=(kc == NT - 1),
                )
            bias = bqkv_sb[:, j * 3 + ty : j * 3 + ty + 1]
            # pair 0's q evacuation runs on ACT (idle until the first exp)
            # while its k evacuations take DVE right after the applies —
            # splitting them lets pair 0's score stream start earliest.
            use_act = j == 0 and ty == 0

            def evac(dst, src, b):
                if use_act:
                    nc.scalar.activation(
                        out=dst, in_=src, func=AF.Identity, bias=b, scale=1.0
                    )
                else:
                    nc.vector.tensor_scalar_add(dst, src, b)

            if ty == 1:
                for h01 in range(2):
                    po = h01 * HD
                    evac(
                        kp_sb[j][h01][po : po + HD, hlf * 512 : (hlf + 1) * 512],
                        acc[po : po + HD, :],
                        bias[po : po + HD, :],
                    )
            else:
                dest = (q_sb, None, v_sb)[ty]
                if dest[j] is None:
                    dest[j] = work.tile(
                        [128, N], bf16, name=f"qkv{j}_{ty}", tag=f"qkv{j}_{ty}"
                    )
                evac(dest[j][:, hlf * 512 : (hlf + 1) * 512], acc, bias)

        def vtrans_pair(j, cr=None):
            """Both heads' v transposed into the persistent vT tiles' v-slots
            by the DMA transpose XBAR (sync HW-DGE queue) — no PE or DVE
            involvement.  cr=(k0,k1) restricts to a kt range so pair 0 can
            transpose the kts covered by each v-half as soon as it lands."""
            k0, k1 = (0, KT) if cr is None else cr
            for h01 in range(2):
                nc.sync.dma_start_transpose(
                    out=vt_sb[j][h01]
                    .rearrange("p (k c) -> p k c", c=128)[:, k0:k1, HD:128],
                    in_=v_sb[j][h01 * HD : (h01 + 1) * HD, k0 * 128 : k1 * 128],
                )

        # ---- attention pair with interleaved filler ----
        # pbs_all[j]: exp outputs per pair, shared so a pair's first score
        # kts can be emitted ("prefixed") during the PREVIOUS pair's cx2
        # block — ACT builds exp inventory while the PE streams context,
        # instead of starting every pair's exp stream cold.
        PREFIX = 2
        pbs_all = [dict() for _ in range(NP)]

        def emit_sc(j, kt):
            for h01 in range(2):
                sc = pA.tile([128, N], f32, name=f"sc{j}_{h01}_{kt}", tag="sc")
                for hlf in range(2):
                    nc.tensor.matmul(
                        sc[:, hlf * 512 : (hlf + 1) * 512],
                        kp_sb[j][h01][:, kt * 128 : (kt + 1) * 128],
                        q_sb[j][:, hlf * 512 : (hlf + 1) * 512],
                    )
                pb = pb_pool.tile(
                    [128, N], bf16, name=f"pb{h01}_{kt}", tag=f"pb{h01}_{kt}"
                )
                nc.scalar.activation(out=pb, in_=sc, func=AF.Exp, scale=0.125)
                pbs_all[j][(h01, kt)] = pb

        def attn_pair(j):
            last = j == NP - 1
            lag = 5 if j == 0 else LAG
            pbs = pbs_all[j]
            cx1 = {}
            cx2 = {}
            # filler units: pair 0 leads with its own v projection +
            # transpose (so its exp stream starts right after q/k);
            # pairs 0-2 then carry pair j+1's qkv halves + v transposes.
            filler = []
            if j == 0:
                for hlf in range(2):
                    filler.append(lambda hlf=hlf: qkv_half(0, 2, hlf))
                filler.append(lambda: vtrans_pair(0))
            if not last:
                jn = j + 1
                for ty in range(3):
                    for hlf in range(2):
                        filler.append(lambda ty=ty, hlf=hlf: qkv_half(jn, ty, hlf))
                filler.append(lambda: vtrans_pair(jn))

            def emit_cx1(kt):
                for h01 in range(2):
                    if kt == 0:
                        cx1[h01] = pX.tile(
                            [128, 512], f32, name=f"cx1_{h01}", tag="cx"
                        )
                    nc.tensor.matmul(
                        cx1[h01],
                        vt_sb[j][h01][:, kt * 128 : (kt + 1) * 128],
                        pbs[(h01, kt)][:, 0:512],
                        start=(kt == 0),
                        stop=(kt == KT - 1),
                    )

            def emit_cx2(kt):
                for h01 in range(2):
                    if kt == 0:
                        pool, tag = (pC, "acc") if last else (pX, "cx")
                        cx2[h01] = pool.tile(
                            [128, 512], f32, name=f"cx2_{h01}", tag=tag
                        )
                    nc.tensor.matmul(
                        cx2[h01],
                        vt_sb[j][h01][:, kt * 128 : (kt + 1) * 128],
                        pbs[(h01, kt)][:, 512:1024],
                        start=(kt == 0),
                        stop=(kt == KT - 1),
                    )

            def normalize(cx, hlf):
                for h01 in range(2):
                    rsb = rsp.tile([HD, 512], f32, name=f"rs{h01}", tag="rs")
                    nc.vector.reciprocal_approx_fast(out=rsb, in_=cx[h01][0:HD, :])
                    nc.vector.tensor_mul(
                        hatt_sb[j][h01 * HD : (h01 + 1) * HD,
                                   hlf * 512 : (hlf + 1) * 512],
                        cx[h01][HD:128, :],
                        rsb,
                    )

            # kt loop: scores + exp lead; first-half context LAGs; filler
            # (next pair's qkv) keeps the PE fed while ACT drains the exp
            # backlog.  Pair 3 interleaves second-half context instead.
            # Pairs 1-3 skip their first PREFIX kts (emitted by the
            # previous pair, below).
            fi = 0
            start = 0 if j == 0 else PREFIX
            for kt in range(start, KT):
                emit_sc(j, kt)
                if kt >= lag:
                    emit_cx1(kt - lag)
                    if last:
                        emit_cx2(kt - lag)
                if (kt > 0 or j > 0) and fi < len(filler):
                    filler[fi]()
                    fi += 1
            for kt in range(KT - lag, KT):
                emit_cx1(kt)
                if last:
                    emit_cx2(kt)
            while fi < len(filler):
                filler[fi]()
                fi += 1
            # prefix the NEXT pair's first score kts here, so its exps run
            # on ACT underneath our cx2 block / normalizes.
            if not last:
                for pkt in range(PREFIX):
                    emit_sc(j + 1, pkt)
            normalize(cx1, 0)
            if not last:
                for kt in range(KT):
                    emit_cx2(kt)
            normalize(cx2, 1)

        for ty in range(2):
            for hlf in range(2):
                qkv_half(0, ty, hlf)
        for j in range(NP):
            attn_pair(j)

        # ---- proj + bias + residual ----
        # residual folded into the accumulation (identity x x), so the
        # evacuation is a copy+bias on ACT — which is idle at the tail —
        # and DVE (busy with pair 3's normalizes) drops out entirely.
        for mt in range(NT):
            ot = outp.tile([128, N], bf16, name=f"ot{mt}", tag="ot")
            for hlf in range(2):
                # rotate accumulator pools by m-tile, earliest-freed first
                ppool, ptag = [(pA, "sc"), (pX, "cx"), (pC, "acc"), (pA, "sc")][mt]
                pp = ppool.tile([128, 512], f32, name=f"pp{mt}_{hlf}", tag=ptag)
                nc.tensor.matmul(
                    pp,
                    ident_sb,
                    x_sb[mt][:, hlf * 512 : (hlf + 1) * 512],
                    start=True,
                    stop=False,
                )
                for kc in range(NT):
                    nc.tensor.matmul(
                        pp,
                        wp_sb[kc][:, mt * 128 : (mt + 1) * 128],
                        hatt_sb[kc][:, hlf * 512 : (hlf + 1) * 512],
                        start=False,
                        stop=(kc == NT - 1),
                    )
                nc.scalar.activation(
                    out=ot[:, hlf * 512 : (hlf + 1) * 512],
                    in_=pp, func=AF.Identity,
                    bias=bproj_sb[:, mt : mt + 1], scale=1.0,
                )
                dq = nc.scalar if hlf == 0 else nc.sync
                dq.dma_start(
                    out=out_dt[mt][:, hlf * 512 : (hlf + 1) * 512],
                    in_=ot[:, hlf * 512 : (hlf + 1) * 512],
                )

    nc.compile()
    return nc


def _get_nc():
    if "nc" not in _CACHE:
        _CACHE["nc"] = _build_program()
    return _CACHE["nc"]


def _host_inputs(x, gn_w, gn_b, qkv_w, qkv_b, proj_w, proj_b):
    f32 = np.float32
    bf = ml_dtypes.bfloat16
    x = np.asarray(x, dtype=f32).reshape(B, C, N)
    gn_w = np.asarray(gn_w, dtype=f32)
    gn_b = np.asarray(gn_b, dtype=f32)
    qkv_w = np.asarray(qkv_w, dtype=f32)
    qkv_b = np.asarray(qkv_b, dtype=f32)
    proj_w = np.asarray(proj_w, dtype=f32)
    proj_b = np.asarray(proj_b, dtype=f32)

    # pair-blocked qkv weights in SBUF layout: [pair, cin 128,
    # (ktile,q|k|v)*cout] — contiguous per partition row for big packets.
    wq = np.zeros((NP, 128, NT * 3, 128), f32)
    bq = np.zeros((128, NP * 3), f32)
    for j in range(NP):
        for ty in range(3):
            rows = qkv_w[ty * 512 + j * 128 : ty * 512 + (j + 1) * 128]  # [128, C]
            for kt in range(NT):
                wq[j, :, kt * 3 + ty, :] = rows[:, kt * 128 : (kt + 1) * 128].T
            bq[:, j * 3 + ty] = qkv_b[ty * 512 + j * 128 : ty * 512 + (j + 1) * 128]
    wq = wq.reshape(NP, 128, NT * 3 * 128)
    # proj weights in SBUF layout: [cin 128, ktile*cout]
    wp = np.ascontiguousarray(
        proj_w.T.reshape(NT, 128, C).transpose(1, 0, 2).reshape(128, NT * C)
    )

    bproj = np.ascontiguousarray(proj_b.reshape(NT, 128).T)
    gnw = np.ascontiguousarray(gn_w.reshape(NT, 128).T)
    gnb = np.ascontiguousarray(gn_b.reshape(NT, 128).T)

    epscol = np.full((128, 1), EPS, f32)
    pad = np.zeros((128, 7), f32)

    gavg = np.ascontiguousarray(
        ((np.arange(128)[:, None] // 16) == (np.arange(128)[None, :] // 16))
        .astype(f32) / (16.0 * N)
    )

    smalls = np.concatenate([bq, bproj, gnw, gnb, epscol, pad], axis=1)
    assert smalls.shape == (128, 32)

    ident2 = np.ascontiguousarray(np.eye(128, dtype=f32))

    common = dict(
        wqkvT=wq.astype(bf), wprojT=wp.astype(bf),
        smalls=np.ascontiguousarray(smalls), gavg=gavg,
        ident2=ident2.astype(bf),
    )
    return [
        dict(common, x=np.ascontiguousarray(x[b]).astype(bf)) for b in range(B)
    ]


def _run(in_maps, trace=False, **kw):
    from concourse.bass_utils import run_bass_kernel_spmd

    nc = _get_nc()
    return run_bass_kernel_spmd(nc, in_maps, list(range(NCORES)), trace=trace, **kw)


def kernel(x, gn_w, gn_b, qkv_w, qkv_b, proj_w, proj_b):
    in_maps = _host_inputs(x, gn_w, gn_b, qkv_w, qkv_b, proj_w, proj_b)
    res = _run(in_maps)
    out = np.stack(
        [np.asarray(res.results[b]["out"]).astype(np.float32) for b in range(B)]
    )
    return out.reshape(B, C, HH, WW)
